# revision 13
# baseline (speedup 1.0000x reference)
"""ChebyshevKAN layer on 8 Trainium2 NeuronCores.

y[b,o] = sum_{i,j} T_j(xn[b,i]) * C[i,o,j],  xn = per-row min/max normalize to [-1,1]

Strategy (per core, batch-sharded 8 ways => 1024 rows/core):
  - normalize x rows on ACT (scale/bias per partition), cast fp16
  - DMA-transpose xn into [i, b] layout tiles
  - Chebyshev recurrence T_{j+1} = 2 xn T_j - T_{j-1} on DVE (fp16, fused
    scalar_tensor_tensor), bt-group-major so the PE never catches up
  - T_0 = 1 contribution precomputed on host as bias[o] = sum_i C[i,o,0],
    seeded into each accumulator by a K=1 rank-1 matmul
  - coeffs pre-transposed on host to [j, i, o] fp16, streamed once per
    bt-group over both HWDGE rings; x loads split across sync/scalar/swdge
    rings so nothing queues ahead of the coeff stream (HWDGE rings are FIFO
    with waits at the head)
  - HW floor is ~237ns per 512-wide fp16 matmul (Ldweights pipelines for
    free); the optimization target is PE idle, not weight reloads
"""

import sys

sys.path.insert(0, "/opt/trn_rl_repo")

import numpy as np

import concourse.bass as bass  # noqa: F401  (bass must import before tile)
import concourse.tile as tile
from concourse import bacc, mybir
from concourse.bass_utils import run_bass_kernel_spmd

NCORES = 8
B_FULL = 8192
B_SH = B_FULL // NCORES  # 1024 rows per core
I_DIM = 1024
O_DIM = 1024
NJ = 9  # degrees 0..8
P = 128
NBT = B_SH // P  # 8 batch tiles per core
NIC = I_DIM // P  # 8 contraction chunks
OT = 512  # output tile width
NOT = O_DIM // OT  # 2
BTG = 4  # batch tiles per PSUM group (BTG * NOT = 8 banks)
NG = NBT // BTG  # 2 groups

_PROGRAM_CACHE = {}


def dedup_ldweights(nc):
    """Remove back-to-back Ldweights with identical stationary APs.

    HW-measured effect is neutral (the PE pipelines weight loads), but it
    drops ~500 instructions of sequencer traffic. Only wait-free,
    update-free Ldweights are dropped, so the semaphore protocol is
    untouched.
    """
    removed = 0
    for blk in nc.main_func.blocks:
        insts = list(blk.instructions)
        drop = []
        prev_key = None
        for inst in insts:
            if isinstance(inst, mybir.InstLdweights):
                key = (
                    str(inst.ins[0]),
                    str(inst.tile_position),
                    str(inst.perf_mode),
                    bool(inst.is_transpose),
                )
                if key == prev_key and not inst.has_wait() and not inst.has_update():
                    drop.append(inst)
                    continue
                prev_key = key
            elif isinstance(inst, mybir.InstMatmult):
                if inst.is_transpose:
                    prev_key = None
            elif inst.engine == mybir.EngineType.PE:
                prev_key = None
        for inst in drop:
            blk.instructions.remove(inst)
        removed += len(drop)
    return removed


def build_program(repeat=1):
    """Build + compile the per-core Bass program (cached).

    repeat>1 wraps the whole body in an on-device loop — used only for
    timing (amortizes host dispatch overhead over `repeat` kernel runs).
    """
    if repeat in _PROGRAM_CACHE:
        return _PROGRAM_CACHE[repeat]

    f16 = mybir.dt.float16
    f32 = mybir.dt.float32

    nc = bacc.Bacc("TRN2", target_bir_lowering=False, debug=False, num_devices=NCORES)
    xs_ext = nc.dram_tensor("xs", [B_SH, I_DIM], f32, kind="ExternalInput")
    cj_ext = nc.dram_tensor("cj", [NJ, I_DIM, O_DIM], f16, kind="ExternalInput")
    bias_ext = nc.dram_tensor("bias", [1, O_DIM], f16, kind="ExternalInput")
    y_ext = nc.dram_tensor("y", [B_SH, O_DIM], f32, kind="ExternalOutput")

    import contextlib

    with tile.TileContext(nc) as tc:
        with (
            tc.tile_pool(name="tall", bufs=1) as tp,
            tc.tile_pool(name="xp", bufs=4) as xpool,
            tc.tile_pool(name="wp", bufs=2) as wpool,
            tc.tile_pool(name="sm", bufs=8) as spool,
            tc.tile_pool(name="cp", bufs=3) as cpool,
            tc.tile_pool(name="op", bufs=2) as opool,
            tc.tile_pool(name="ps", bufs=8, space="PSUM") as pspool,
            tc.For_i(0, repeat, 1) if repeat > 1 else contextlib.nullcontext(),
        ):
            # T_all[:, j-1, ic, bt, :] holds T_j in transposed [i, b] layout
            T_all = tp.tile([P, NJ - 1, NIC, NBT, P], f16)

            ones_row = tp.tile([1, P], f16)
            nc.vector.memset(ones_row, 1.0)
            bias16 = tp.tile([1, O_DIM], f16)
            nc.sync.dma_start(out=bias16, in_=bias_ext[:, :])

            # x loads: first 4 bt on the two HWDGE rings (ahead of coeffs,
            # they are needed first), rest on the SWDGE ring (lands by ~12us,
            # well before needed)
            x_sbs = {}
            for bt in range(NBT):
                x_sb = xpool.tile([P, I_DIM], f32, name=f"x_{bt}", tag="x_sb")
                nc.gpsimd.dma_start(out=x_sb, in_=xs_ext[bt * P : (bt + 1) * P, :])
                x_sbs[bt] = x_sb

            # coeff slabs: pre-allocate in consumption order; pre-issue the
            # first group's leading slabs so the HWDGE rings stream them
            # immediately behind the 4 x tiles
            c_tiles = {}
            for g in range(NG):
                for j in range(1, NJ):
                    c_tiles[(g, j)] = cpool.tile(
                        [P, NIC, O_DIM], f16, name=f"c_{g}_{j}", tag="c_sb"
                    )

            def issue_c(g, j, chunks=4):
                # fine-granularity completion sems let matmuls start on the
                # first ic chunks while later ones are still in flight;
                # chunks interleave across both rings in consumption order
                full = cj_ext[j, :, :].rearrange("(ic p) o -> p ic o", p=P)
                t = c_tiles[(g, j)]
                q = NIC // chunks
                for k in range(chunks):
                    eng = (nc.sync, nc.scalar)[k % 2]
                    eng.dma_start(
                        out=t[:, k * q : (k + 1) * q, :],
                        in_=full[:, k * q : (k + 1) * q, :],
                    )

            issue_c(0, 1, chunks=8)
            for j in range(2, 4):
                issue_c(0, j)

            # ---- Phase A: normalize + transpose, per batch tile ----
            for bt in range(NBT):
                x_sb = x_sbs[bt]
                mx = spool.tile([P, 1], f32)
                mn = spool.tile([P, 1], f32)
                nc.vector.tensor_reduce(
                    out=mx, in_=x_sb, op=mybir.AluOpType.max, axis=mybir.AxisListType.X
                )
                nc.vector.tensor_reduce(
                    out=mn, in_=x_sb, op=mybir.AluOpType.min, axis=mybir.AxisListType.X
                )
                st2 = spool.tile([P, 2], f32)
                s = st2[:, 0:1]
                t = st2[:, 1:2]
                rng = spool.tile([P, 1], f32)
                nc.vector.tensor_sub(out=rng, in0=mx, in1=mn)
                nc.vector.reciprocal(out=s, in_=rng)
                nc.vector.tensor_scalar_mul(s, s, 2.0)
                # t = (mn * -1) * s - 1
                nc.vector.scalar_tensor_tensor(
                    out=t, in0=mn, scalar=-1.0, in1=s,
                    op0=mybir.AluOpType.mult, op1=mybir.AluOpType.mult,
                )
                nc.vector.tensor_scalar_add(t, t, -1.0)

                xt16 = wpool.tile([P, I_DIM], f16)
                nc.scalar.activation(
                    out=xt16, in_=x_sb,
                    func=mybir.ActivationFunctionType.Identity,
                    bias=t, scale=s,
                )
                # T_1 = xn, transposed into [i, b] tiles
                eng = (nc.sync, nc.scalar)[bt % 2]
                eng.dma_start_transpose(out=T_all[:, 0, :, bt, :], in_=xt16)

            # ---- Phase B: Chebyshev recurrence, bt-group-major so the PE
            # (which consumes group 0 for the first ~half of the matmul
            # stream) never overtakes the DVE ----
            for g in range(NG):
                for j in range(2, NJ):
                    for bt in range(g * BTG, (g + 1) * BTG):
                        t1 = T_all[:, 0, :, bt, :]
                        tprev = T_all[:, j - 2, :, bt, :]
                        cur = T_all[:, j - 1, :, bt, :]
                        prod = wpool.tile([P, NIC, P], f16)
                        nc.vector.scalar_tensor_tensor(
                            out=prod, in0=tprev, scalar=2.0, in1=t1,
                            op0=mybir.AluOpType.mult, op1=mybir.AluOpType.mult,
                        )
                        if j == 2:
                            nc.vector.tensor_scalar_add(cur, prod, -1.0)
                        else:
                            nc.vector.tensor_sub(
                                out=cur, in0=prod, in1=T_all[:, j - 3, :, bt, :]
                            )

            # ---- Phase C: matmuls, coeffs streamed once per bt-group ----
            for g in range(NG):
                psums = [
                    [
                        pspool.tile([P, OT], f32, name=f"ps{g}_{bt}_{ot}", tag="psacc")
                        for ot in range(NOT)
                    ]
                    for bt in range(BTG)
                ]
                # seed accumulators with the T_0 bias via K=1 rank-1 matmul
                for bt in range(BTG):
                    for ot in range(NOT):
                        nc.tensor.matmul(
                            psums[bt][ot],
                            lhsT=ones_row,
                            rhs=bias16[:, ot * OT : (ot + 1) * OT],
                            start=True,
                            stop=False,
                        )
                for j in range(1, NJ):
                    if not (g == 0 and j < 4):
                        issue_c(g, j)
                    c_sb = c_tiles[(g, j)]
                    if j == NJ - 1:
                        # last slab bt-major: each PSUM bank stops after its
                        # own 16 matmuls, so drain overlaps the remaining MMs
                        order = [(i, b) for b in range(BTG) for i in range(NIC)]
                    else:
                        order = [(i, b) for i in range(NIC) for b in range(BTG)]
                    for ic, bt in order:
                        for ot in range(NOT):
                            nc.tensor.matmul(
                                psums[bt][ot],
                                lhsT=T_all[:, j - 1, ic, g * BTG + bt, :],
                                rhs=c_sb[:, ic, ot * OT : (ot + 1) * OT],
                                start=False,
                                stop=(j == NJ - 1 and ic == NIC - 1),
                            )
                for bt in range(BTG):
                    row0 = (g * BTG + bt) * P
                    for ot in range(NOT):
                        o_sb = opool.tile([P, OT], f32)
                        # alternate copy engines so the final drain is 2-wide
                        if ot == 0:
                            nc.scalar.copy(out=o_sb, in_=psums[bt][ot])
                        else:
                            nc.vector.tensor_copy(o_sb, psums[bt][ot])
                        # group 0 drains on the idle SWDGE ring so the HWDGE
                        # rings keep streaming coeffs; the tail group uses the
                        # SP ring (idle by then; ACT is busy with psum copies)
                        eng = nc.gpsimd if g == 0 else nc.sync
                        eng.dma_start(
                            out=y_ext[row0 : row0 + P, ot * OT : (ot + 1) * OT],
                            in_=o_sb,
                        )

    nc.compile()
    dedup_ldweights(nc)
    _PROGRAM_CACHE[repeat] = nc
    return nc


def host_prep(x, cheby_coeffs):
    """Host-side layout prep: coeffs -> [j, i, o] fp16, bias, x row shards."""
    cf = np.asarray(cheby_coeffs)
    cj = np.ascontiguousarray(np.transpose(cf, (2, 0, 1))).astype(np.float16)
    bias = cf[:, :, 0].sum(axis=0, dtype=np.float64).astype(np.float16)[None, :]
    x = np.asarray(x, dtype=np.float32).reshape(B_FULL, I_DIM)
    in_maps = [
        {"xs": x[i * B_SH : (i + 1) * B_SH], "cj": cj, "bias": bias}
        for i in range(NCORES)
    ]
    return in_maps


def kernel(x, cheby_coeffs):
    nc = build_program(1)
    in_maps = host_prep(x, cheby_coeffs)
    res = run_bass_kernel_spmd(nc, in_maps, list(range(NCORES)))
    return np.concatenate([r["y"] for r in res.results], axis=0)
